# revision 14
# baseline (speedup 1.0000x reference)
"""Trainium2 Bass kernel for CRF forward-algorithm loss (logsumexp scan).

Exp-domain matmul recurrence:
    u_t = exp(emit_t - C) * (P @ u_{t-1}),  P = exp(trans), u kept [tags, batch]

v3: single merged batch group (N=16 moving) — the PE is matmul-issue-bound
(~25 ns/instr regardless of N<=32), so 16 matmuls/step beats v2's 32.
Per-chain (j) DVE multiplies staggered right after each accumulation chain
so only the last chunk's multiply latency is exposed.

  - Weights fp8e4m3 (stationary), moving bf16: rel err ~1e-4.
  - Emissions DMA'd 8 steps per transfer, exp() applied per 8-step tile.
  - Renorm every 64 steps (exact bookkeeping via stored z, log on host).

Sharding: data-parallel over batch, 16 per core on 8 cores, host sums.
"""

import numpy as np
import ml_dtypes

import concourse.bass as bass
import concourse.mybir as mybir
import concourse.tile as tile
from concourse import bacc
from concourse.bass_utils import run_bass_kernel_spmd

T = 512
S = 512
B = 128
NCORES = 8
BL = B // NCORES   # 16 per core
TC = 4
START = 510
STOP = 511
C = 7.0
R = 64
NREN = S // R      # 8
DG = 8             # steps per DMA group

F32 = mybir.dt.float32
BF16 = mybir.dt.bfloat16
FP8 = mybir.dt.float8e4


def _dedup_ldweights(nc):
    removed = 0
    for blk in nc.m.functions[0].blocks:
        insts = blk.instructions
        last_w = None
        to_del = []
        for inst in insts:
            tn = type(inst).__name__
            if tn == "InstLdweights":
                sig = repr(inst.ins[0])
                si = inst.sync_info
                clean = si is None or (
                    len(si.on_wait) == 0 and len(si.on_update) == 0
                )
                if sig == last_w and clean:
                    to_del.append(inst)
                else:
                    last_w = sig
        for inst in to_del:
            insts.remove(inst)
            removed += 1
    return removed


def _build_program():
    nc = bacc.Bacc(
        "TRN2",
        target_bir_lowering=False,
        debug=False,
        enable_asserts=False,
        num_devices=NCORES,
    )

    pt_d = nc.dram_tensor("pt", [128, TC * TC * 128], FP8, kind="ExternalInput")
    pstop_d = nc.dram_tensor("pstop", [128, TC], BF16, kind="ExternalInput")
    u0_d = nc.dram_tensor("u0", [128, TC * BL], BF16, kind="ExternalInput")
    em_d = nc.dram_tensor("emt", [S // DG, 128, DG * TC * BL], F32,
                          kind="ExternalInput")
    fin_d = nc.dram_tensor("fin", [1, BL], F32, kind="ExternalOutput")
    zs_d = nc.dram_tensor("zs", [1, NREN * BL], F32, kind="ExternalOutput")

    with tile.TileContext(nc) as tc:
        with (
            tc.tile_pool(name="singles", bufs=1) as singles,
            tc.tile_pool(name="empool", bufs=3) as empool,
            tc.tile_pool(name="ehpool", bufs=3) as ehpool,
            tc.tile_pool(name="upool", bufs=2) as upool,
            tc.tile_pool(name="rnpool", bufs=2) as rnpool,
            tc.tile_pool(name="pspool", bufs=2, space="PSUM") as pspool,
            tc.tile_pool(name="pzpool", bufs=1, space="PSUM") as pzpool,
        ):
            ptsb = singles.tile([128, TC * TC * 128], FP8)
            nc.sync.dma_start(out=ptsb, in_=pt_d[:, :])
            pstop_sb = singles.tile([128, TC], BF16)
            nc.sync.dma_start(out=pstop_sb, in_=pstop_d[:, :])
            u = [None] * 2
            for pair in range(2):
                up = upool.tile([128, 2 * BL], BF16, name=f"u{pair}",
                                tag=f"u{pair}")
                nc.sync.dma_start(
                    out=up, in_=u0_d[:, 2 * pair * BL : (2 * pair + 2) * BL]
                )
                u[pair] = up
            ones_sb = singles.tile([128, 1], BF16)
            nc.vector.memset(ones_sb, 1.0)
            negc_sb = singles.tile([128, 1], F32)
            nc.vector.memset(negc_sb, -C)
            zs_sb = singles.tile([1, NREN * BL], F32)

            def fetch(gi):
                em8 = empool.tile([128, DG * TC * BL], F32, name="em8",
                                  tag="em")
                nc.sync.dma_start(out=em8, in_=em_d[gi])
                eh = ehpool.tile([128, DG * TC * BL], F32, name="eh8",
                                 tag="eh")
                nc.scalar.activation(
                    eh, em8, mybir.ActivationFunctionType.Exp,
                    bias=negc_sb, scale=1.0,
                )
                return eh

            eh8 = fetch(0)
            eh_next = None
            for t in range(S):
                s = t % DG
                if s == 0 and t > 0:
                    eh8 = eh_next
                if s == 1 and t // DG + 1 < S // DG:
                    eh_next = fetch(t // DG + 1)
                u_new = [None] * 2
                for pair in range(2):
                    ps = pspool.tile([128, 2 * BL], F32, name=f"ps{pair}",
                                     tag=f"ps{pair}")
                    for jj in range(2):
                        j = 2 * pair + jj
                        for i in range(TC):
                            w = ptsb[:, (i * TC + j) * 128 : (i * TC + j + 1) * 128]
                            nc.tensor.matmul(
                                ps[:, jj * BL : (jj + 1) * BL], w,
                                u[i // 2][:, (i % 2) * BL : (i % 2 + 1) * BL],
                                start=(i == 0), stop=(i == TC - 1),
                                skip_group_check=True,
                            )
                    up = upool.tile([128, 2 * BL], BF16, name=f"u{pair}",
                                    tag=f"u{pair}")
                    off = s * TC * BL + 2 * pair * BL
                    nc.vector.tensor_mul(up, ps, eh8[:, off : off + 2 * BL])
                    u_new[pair] = up

                if t % R == R - 1:
                    r = t // R
                    zp = pzpool.tile([1, BL], F32, name="zp", tag="z")
                    for i in range(TC):
                        nc.tensor.matmul(
                            zp, ones_sb,
                            u_new[i // 2][:, (i % 2) * BL : (i % 2 + 1) * BL],
                            start=(i == 0), stop=(i == TC - 1),
                            skip_group_check=True,
                        )
                    nc.vector.tensor_copy(
                        zs_sb[0:1, r * BL : (r + 1) * BL], zp
                    )
                    zr = rnpool.tile([1, BL], F32, name="zr", tag="zr")
                    nc.vector.reciprocal(zr, zp)
                    zb = rnpool.tile([128, 2 * BL], F32, name="zb", tag="zb")
                    nc.gpsimd.partition_broadcast(zb[:, 0:BL], zr)
                    nc.gpsimd.partition_broadcast(zb[:, BL : 2 * BL], zr)
                    for pair in range(2):
                        nc.gpsimd.tensor_mul(u_new[pair], u_new[pair], zb)
                u = u_new

            fin_sb = singles.tile([1, BL], F32)
            finp = pzpool.tile([1, BL], F32, name="finp", tag="z")
            for i in range(TC):
                nc.tensor.matmul(
                    finp, pstop_sb[:, i : i + 1],
                    u[i // 2][:, (i % 2) * BL : (i % 2 + 1) * BL],
                    start=(i == 0), stop=(i == TC - 1),
                    skip_group_check=True,
                )
            nc.vector.tensor_copy(fin_sb, finp)
            nc.sync.dma_start(out=fin_d[0:1, :], in_=fin_sb)
            nc.sync.dma_start(out=zs_d[0:1, :], in_=zs_sb)

    n = _dedup_ldweights(nc)
    nc._ldw_removed = n
    nc.compile()
    return nc


def _prep_inputs(emissions, transitions):
    bf = ml_dtypes.bfloat16
    P = np.exp(transitions.astype(np.float32))
    PT = np.ascontiguousarray(P.T)                      # [prev, next]
    pt_host = np.ascontiguousarray(
        PT.reshape(TC, 128, TC, 128).transpose(1, 0, 2, 3)
    ).reshape(128, TC * TC * 128).astype(ml_dtypes.float8_e4m3)
    pstop = np.exp(transitions[STOP].astype(np.float32))
    pstop_host = np.ascontiguousarray(pstop.reshape(TC, 128).T).astype(bf)
    u0_host = np.zeros((128, TC * BL), dtype=bf)
    u0_host[START % 128, (START // 128) * BL : (START // 128 + 1) * BL] = 1.0

    in_maps = []
    for c in range(NCORES):
        sh = emissions[c * BL : (c + 1) * BL]           # [BL, S, T]
        # emt[gi, k, ((s, i, b))] = sh[b, 8*gi+s, 128*i+k]
        a = sh.transpose(1, 2, 0)                       # [t, n, b]
        a = a.reshape(S // DG, DG, TC, 128, BL)         # [gi, s, i, k, b]
        emt = np.ascontiguousarray(a.transpose(0, 3, 1, 2, 4)).reshape(
            S // DG, 128, DG * TC * BL
        ).astype(np.float32)
        in_maps.append({"pt": pt_host, "pstop": pstop_host, "u0": u0_host,
                        "emt": emt})
    return in_maps


def _loss_from_outputs(results):
    total = 0.0
    for res in results:
        fin = np.asarray(res["fin"], np.float64).reshape(BL)
        zs = np.asarray(res["zs"], np.float64).reshape(NREN, BL)
        loss_b = np.log(fin) + np.log(zs).sum(axis=0) + S * C
        total += loss_b.sum()
    return np.float32(total)


def _run(inputs, **kwargs):
    emissions = np.asarray(inputs["inputs"], dtype=np.float32)
    transitions = np.asarray(inputs["transitions"], dtype=np.float32)
    assert emissions.shape == (B, S, T), emissions.shape
    nc = _build_program()
    in_maps = _prep_inputs(emissions, transitions)
    res = run_bass_kernel_spmd(nc, in_maps, core_ids=list(range(NCORES)), **kwargs)
    return _loss_from_outputs(res.results), res


def kernel(**inputs) -> np.ndarray:
    out, _ = _run(inputs)
    return out


# revision 15
# speedup vs baseline: 1.1346x; 1.1346x over previous
"""Trainium2 Bass kernel for CRF forward-algorithm loss (logsumexp scan).

Exp-domain matmul recurrence:
    u_t = exp(emit_t - C) * (P @ u_{t-1}),  P = exp(trans), u kept [tags, batch]

v3: single merged batch group (N=16 moving) — the PE is matmul-issue-bound
(~25 ns/instr regardless of N<=32), so 16 matmuls/step beats v2's 32.
Per-chain (j) DVE multiplies staggered right after each accumulation chain
so only the last chunk's multiply latency is exposed.

  - Weights fp8e4m3 (stationary), moving bf16: rel err ~1e-4.
  - Emissions DMA'd 8 steps per transfer, exp() applied per 8-step tile.
  - Renorm every 64 steps (exact bookkeeping via stored z, log on host).

Sharding: data-parallel over batch, 16 per core on 8 cores, host sums.
"""

import numpy as np
import ml_dtypes

import concourse.bass as bass
import concourse.mybir as mybir
import concourse.tile as tile
from concourse import bacc
from concourse.bass_utils import run_bass_kernel_spmd

T = 512
S = 512
B = 128
NCORES = 8
BL = B // NCORES   # 16 per core
TC = 4
START = 510
STOP = 511
C = 7.0
R = 256
NREN = S // R      # 2
DG = 8             # steps per DMA group

F32 = mybir.dt.float32
BF16 = mybir.dt.bfloat16
FP8 = mybir.dt.float8e4


def _dedup_ldweights(nc):
    removed = 0
    for blk in nc.m.functions[0].blocks:
        insts = blk.instructions
        last_w = None
        to_del = []
        for inst in insts:
            tn = type(inst).__name__
            if tn == "InstLdweights":
                sig = repr(inst.ins[0])
                si = inst.sync_info
                clean = si is None or (
                    len(si.on_wait) == 0 and len(si.on_update) == 0
                )
                if sig == last_w and clean:
                    to_del.append(inst)
                else:
                    last_w = sig
        for inst in to_del:
            insts.remove(inst)
            removed += 1
    return removed


def _build_program():
    nc = bacc.Bacc(
        "TRN2",
        target_bir_lowering=False,
        debug=False,
        enable_asserts=False,
        num_devices=NCORES,
    )

    pt_d = nc.dram_tensor("pt", [128, TC * TC * 128], FP8, kind="ExternalInput")
    pstop_d = nc.dram_tensor("pstop", [128, TC], BF16, kind="ExternalInput")
    u0_d = nc.dram_tensor("u0", [128, TC * BL], BF16, kind="ExternalInput")
    em_d = nc.dram_tensor("emt", [S // DG, 128, DG * TC * BL], F32,
                          kind="ExternalInput")
    fin_d = nc.dram_tensor("fin", [1, BL], F32, kind="ExternalOutput")
    zs_d = nc.dram_tensor("zs", [1, NREN * BL], F32, kind="ExternalOutput")

    with tile.TileContext(nc) as tc:
        with (
            tc.tile_pool(name="singles", bufs=1) as singles,
            tc.tile_pool(name="empool", bufs=3) as empool,
            tc.tile_pool(name="ehpool", bufs=3) as ehpool,
            tc.tile_pool(name="upool", bufs=2) as upool,
            tc.tile_pool(name="rnpool", bufs=2) as rnpool,
            tc.tile_pool(name="pspool", bufs=2, space="PSUM") as pspool,
            tc.tile_pool(name="pzpool", bufs=1, space="PSUM") as pzpool,
        ):
            ptsb = singles.tile([128, TC * TC * 128], FP8)
            nc.sync.dma_start(out=ptsb, in_=pt_d[:, :])
            pstop_sb = singles.tile([128, TC], BF16)
            nc.sync.dma_start(out=pstop_sb, in_=pstop_d[:, :])
            u = [None] * 2
            for pair in range(2):
                up = upool.tile([128, 2 * BL], BF16, name=f"u{pair}",
                                tag=f"u{pair}")
                nc.sync.dma_start(
                    out=up, in_=u0_d[:, 2 * pair * BL : (2 * pair + 2) * BL]
                )
                u[pair] = up
            ones_sb = singles.tile([128, 1], BF16)
            nc.vector.memset(ones_sb, 1.0)
            onesr_sb = singles.tile([1, 128], F32)
            nc.vector.memset(onesr_sb, 1.0)
            negc_sb = singles.tile([128, 1], F32)
            nc.vector.memset(negc_sb, -C)
            zs_sb = singles.tile([1, NREN * BL], F32)

            def fetch(gi):
                em8 = empool.tile([128, DG * TC * BL], F32, name="em8",
                                  tag="em")
                nc.sync.dma_start(out=em8, in_=em_d[gi])
                eh = ehpool.tile([128, DG * TC * BL], F32, name="eh8",
                                 tag="eh")
                nc.scalar.activation(
                    eh, em8, mybir.ActivationFunctionType.Exp,
                    bias=negc_sb, scale=1.0,
                )
                return eh

            eh8 = fetch(0)
            eh_next = None
            for t in range(S):
                s = t % DG
                if s == 0 and t > 0:
                    eh8 = eh_next
                if s == 1 and t // DG + 1 < S // DG:
                    eh_next = fetch(t // DG + 1)
                u_new = [None] * 2
                for pair in range(2):
                    ps = pspool.tile([128, 2 * BL], F32, name=f"ps{pair}",
                                     tag=f"ps{pair}")
                    for jj in range(2):
                        j = 2 * pair + jj
                        for i in range(TC):
                            w = ptsb[:, (i * TC + j) * 128 : (i * TC + j + 1) * 128]
                            nc.tensor.matmul(
                                ps[:, jj * BL : (jj + 1) * BL], w,
                                u[i // 2][:, (i % 2) * BL : (i % 2 + 1) * BL],
                                start=(i == 0), stop=(i == TC - 1),
                                skip_group_check=True,
                            )
                    up = upool.tile([128, 2 * BL], BF16, name=f"u{pair}",
                                    tag=f"u{pair}")
                    off = s * TC * BL + 2 * pair * BL
                    nc.vector.tensor_mul(up, ps, eh8[:, off : off + 2 * BL])
                    u_new[pair] = up

                if t % R == R - 1:
                    r = t // R
                    zp = pzpool.tile([1, BL], F32, name="zp", tag="z")
                    for i in range(TC):
                        nc.tensor.matmul(
                            zp, ones_sb,
                            u_new[i // 2][:, (i % 2) * BL : (i % 2 + 1) * BL],
                            start=(i == 0), stop=(i == TC - 1),
                            skip_group_check=True,
                        )
                    nc.vector.tensor_copy(
                        zs_sb[0:1, r * BL : (r + 1) * BL], zp
                    )
                    zr2 = rnpool.tile([1, 2 * BL], F32, name="zr2", tag="zr")
                    nc.vector.reciprocal(zr2[0:1, 0:BL], zp)
                    nc.vector.reciprocal(zr2[0:1, BL : 2 * BL], zp)
                    zbp = pzpool.tile([128, 2 * BL], F32, name="zbp", tag="zb")
                    nc.tensor.matmul(zbp, onesr_sb, zr2, start=True, stop=True,
                                     skip_group_check=True)
                    for pair in range(2):
                        nc.vector.tensor_mul(u_new[pair], u_new[pair], zbp)
                u = u_new

            fin_sb = singles.tile([1, BL], F32)
            finp = pzpool.tile([1, BL], F32, name="finp", tag="z")
            for i in range(TC):
                nc.tensor.matmul(
                    finp, pstop_sb[:, i : i + 1],
                    u[i // 2][:, (i % 2) * BL : (i % 2 + 1) * BL],
                    start=(i == 0), stop=(i == TC - 1),
                    skip_group_check=True,
                )
            nc.vector.tensor_copy(fin_sb, finp)
            nc.sync.dma_start(out=fin_d[0:1, :], in_=fin_sb)
            nc.sync.dma_start(out=zs_d[0:1, :], in_=zs_sb)

    n = _dedup_ldweights(nc)
    nc._ldw_removed = n
    nc.compile()
    return nc


def _prep_inputs(emissions, transitions):
    bf = ml_dtypes.bfloat16
    P = np.exp(transitions.astype(np.float32))
    PT = np.ascontiguousarray(P.T)                      # [prev, next]
    pt_host = np.ascontiguousarray(
        PT.reshape(TC, 128, TC, 128).transpose(1, 0, 2, 3)
    ).reshape(128, TC * TC * 128).astype(ml_dtypes.float8_e4m3)
    pstop = np.exp(transitions[STOP].astype(np.float32))
    pstop_host = np.ascontiguousarray(pstop.reshape(TC, 128).T).astype(bf)
    u0_host = np.zeros((128, TC * BL), dtype=bf)
    u0_host[START % 128, (START // 128) * BL : (START // 128 + 1) * BL] = 1.0

    in_maps = []
    for c in range(NCORES):
        sh = emissions[c * BL : (c + 1) * BL]           # [BL, S, T]
        # emt[gi, k, ((s, i, b))] = sh[b, 8*gi+s, 128*i+k]
        a = sh.transpose(1, 2, 0)                       # [t, n, b]
        a = a.reshape(S // DG, DG, TC, 128, BL)         # [gi, s, i, k, b]
        emt = np.ascontiguousarray(a.transpose(0, 3, 1, 2, 4)).reshape(
            S // DG, 128, DG * TC * BL
        ).astype(np.float32)
        in_maps.append({"pt": pt_host, "pstop": pstop_host, "u0": u0_host,
                        "emt": emt})
    return in_maps


def _loss_from_outputs(results):
    total = 0.0
    for res in results:
        fin = np.asarray(res["fin"], np.float64).reshape(BL)
        zs = np.asarray(res["zs"], np.float64).reshape(NREN, BL)
        loss_b = np.log(fin) + np.log(zs).sum(axis=0) + S * C
        total += loss_b.sum()
    return np.float32(total)


def _run(inputs, **kwargs):
    emissions = np.asarray(inputs["inputs"], dtype=np.float32)
    transitions = np.asarray(inputs["transitions"], dtype=np.float32)
    assert emissions.shape == (B, S, T), emissions.shape
    nc = _build_program()
    in_maps = _prep_inputs(emissions, transitions)
    res = run_bass_kernel_spmd(nc, in_maps, core_ids=list(range(NCORES)), **kwargs)
    return _loss_from_outputs(res.results), res


def kernel(**inputs) -> np.ndarray:
    out, _ = _run(inputs)
    return out
